# revision 12
# baseline (speedup 1.0000x reference)
"""Trainium2 Bass kernel: parity-polynomial segment_reduce.

Reference math:
    spins = 1 - 2*bits                                   # {-1,+1}
    parities[b,t] = prod_o spins_pad[b, idx_pad[t,o]]    # [B, T]
    out[b] = parities[b] @ theta

Every parity factor is (-1)^{bit}, so
    out[b] = sum_t theta[t] * (-1)^{popcount(key[b] & mask[t])}
with key[b] = sum_i bits[b,i]<<i and mask[t] = XOR-fold of (1<<idx_pad[t,o])
(the pad index NUM_BITS maps to a constant-one column, i.e. contributes no bit;
a repeated index squares to +1, which XOR-folding reproduces).

For this problem idx_pad only references bits 0..11, so every mask < 4096 and
out[b] = f(key12[b]) where f = WHT_4096(theta_spread) — a 4096-point
Walsh-Hadamard transform of theta scattered by mask.  On device (per core,
batch-sharded 512 rows):

  1. WHT via the Kronecker split H_4096 = H_128 (x) H_32:
         F[p,c] = (H128 @ Theta @ H32)[p,c],  Theta[q,d] = theta_spread[q*32+d]
     -> two small PE matmuls (theta passed pre-transposed to avoid an
        on-device transpose).
  2. Per-row keys p_b (bits 5..11) and c_b (bits 0..4) via PE matmuls whose
     stationary weights are pow-2 columns replicated across the free dim --
     this both computes the key and broadcasts it across partitions.
  3. One-hot rows by comparing the broadcast keys against an iota column
     (tensor_scalar is_equal with a per-partition scalar).
  4. Gather F rows with a one-hot matmul, mask columns with the c one-hot,
     and column-reduce with a ones-vector matmul:
         out[b] = sum_c (F^T @ onehot_p)[c,b] * onehot_c[c,b] = F[p_b, c_b].

Host does only sharding, dtype/layout staging, and the index bookkeeping
(mask XOR-fold + theta scatter).  All theta- and bit-dependent arithmetic
runs on device.
"""

import numpy as np

B, NUM_BITS, ORDER = 4096, 32, 12
N_CORES = 8
B_LOCAL = B // N_CORES          # 512
KEYS = 1 << ORDER               # 4096
P_DIM, C_DIM = 128, 32          # KEYS = P_DIM * C_DIM ; p = key>>5, c = key&31
P_BITS, C_BITS = 7, 5
PK32_COLS = 322   # thetaT(128) | h32(32) | wp(128) | wc(32) | iotac(1) | ones(1)
PK128_COLS = 129  # h128(128) | iotap(1)

_STATE = {}


def _sylvester(n):
    """H[i,j] = (-1)^popcount(i&j), Sylvester ordering."""
    h = np.array([[1.0]], dtype=np.float32)
    while h.shape[0] < n:
        h = np.block([[h, h], [h, -h]])
    return np.ascontiguousarray(h, dtype=np.float32)


def _build_module():
    import concourse.mybir as mybir
    import concourse.tile as tile
    from concourse import bacc

    f32 = mybir.dt.float32
    nc = bacc.Bacc(
        "TRN2",
        target_bir_lowering=False,
        debug=False,
        enable_asserts=True,
        num_devices=N_CORES,
    )

    # Packed inputs: few DMAs -> few DMA semaphores per consumer instruction
    # (walrus rejects instructions with too many sync waits) and less DMA
    # first-byte overhead.  bitsT carries an extra constant-ones row (row 32)
    # whose weight in wp/wc is -j, so the key matmuls directly produce
    # key(b) - j and the one-hot compare is against an immediate 0.0 (the
    # pointer-scalar tensor_scalar form has no room for any sync wait).
    #   pk33 [33, 322]: thetaT | h32 | wp_aug | wc_aug | - | ones
    #   pk128 [128, 128]: h128
    bitsT = nc.dram_tensor("bitst", [NUM_BITS + 1, B_LOCAL], f32, kind="ExternalInput").ap()
    pk33 = nc.dram_tensor("pk33", [NUM_BITS + 1, PK32_COLS], f32, kind="ExternalInput").ap()
    pk128 = nc.dram_tensor("pk128", [P_DIM, P_DIM], f32, kind="ExternalInput").ap()
    out = nc.dram_tensor("out", [1, B_LOCAL], f32, kind="ExternalOutput").ap()

    with tile.TileContext(nc) as tc:
        with (
            tc.tile_pool(name="sb", bufs=1) as sb,
            tc.tile_pool(name="ps", bufs=1, space="PSUM") as ps,
        ):
            t_bitsT = sb.tile([NUM_BITS + 1, B_LOCAL], f32)
            nc.sync.dma_start(out=t_bitsT, in_=bitsT)
            t_pk33 = sb.tile([NUM_BITS + 1, PK32_COLS], f32)
            nc.sync.dma_start(out=t_pk33, in_=pk33)
            t_pk128 = sb.tile([P_DIM, P_DIM], f32)
            nc.sync.dma_start(out=t_pk128, in_=pk128)

            t_thetaT = t_pk33[0:C_DIM, 0:128]
            t_h32 = t_pk33[0:C_DIM, 128:160]
            t_wp = t_pk33[:, 160:288]
            t_wc = t_pk33[:, 288:320]
            t_ones = t_pk33[0:C_DIM, 321:322]
            t_h128 = t_pk128

            # --- WHT of theta_spread: F = H128 @ Theta @ H32 ---
            # PE-order discipline: each matmul must introduce at most ONE new
            # cross-engine wait (the ISA load-weights slot is tiny), so PE
            # observes each DMA semaphore via a cheap warm-up matmul first and
            # every PSUM->SBUF staging copy runs on DVE only.
            # G[q,c] = sum_d Theta[q,d] H32[d,c]   (contraction over d on partitions)
            p_G = ps.tile([P_DIM, C_DIM], f32)
            nc.tensor.matmul(p_G, t_thetaT, t_h32)          # waits: pk33 DMA
            p_warm = ps.tile([1, 1], f32)
            nc.tensor.matmul(p_warm, t_h128[:, 0:1], t_h128[:, 0:1])  # waits: pk128 DMA
            t_G = sb.tile([P_DIM, C_DIM], f32)
            nc.vector.tensor_copy(t_G, p_G)
            # F[p,c] = sum_q H128[q,p] G[q,c]  (H128 symmetric)
            p_F = ps.tile([P_DIM, C_DIM], f32)
            nc.tensor.matmul(p_F, t_h128, t_G)              # waits: DVE only
            t_F = sb.tile([P_DIM, C_DIM], f32)
            nc.vector.tensor_copy(t_F, p_F)

            # --- keys minus partition index, via replicated pow2 columns plus
            # a -iota weight on the constant-ones bit row ---
            p_bp = ps.tile([P_DIM, B_LOCAL], f32)
            nc.tensor.matmul(p_bp, t_wp, t_bitsT)  # [j, b] = p_key(b) - j
            t_ohp = sb.tile([P_DIM, B_LOCAL], f32)
            nc.vector.tensor_scalar(
                out=t_ohp,
                in0=p_bp,
                scalar1=0.0,
                scalar2=None,
                op0=mybir.AluOpType.is_equal,
            )
            p_bc = ps.tile([C_DIM, B_LOCAL], f32)
            nc.tensor.matmul(p_bc, t_wc, t_bitsT)  # [j, b] = c_key(b) - j
            t_ohc = sb.tile([C_DIM, B_LOCAL], f32)
            nc.vector.tensor_scalar(
                out=t_ohc,
                in0=p_bc,
                scalar1=0.0,
                scalar2=None,
                op0=mybir.AluOpType.is_equal,
            )

            # --- gather + reduce:  out[b] = F[p_b, c_b] ---
            p_o1 = ps.tile([C_DIM, B_LOCAL], f32)
            nc.tensor.matmul(p_o1, t_F, t_ohp)      # o1[c,b] = F[p_b, c]
            t_prod = sb.tile([C_DIM, B_LOCAL], f32)
            nc.vector.tensor_mul(t_prod, p_o1, t_ohc)
            p_out = ps.tile([1, B_LOCAL], f32)
            nc.tensor.matmul(p_out, t_ones, t_prod)  # column sums
            t_out = sb.tile([1, B_LOCAL], f32)
            nc.vector.tensor_copy(t_out, p_out)
            nc.sync.dma_start(out=out, in_=t_out)

    nc.compile()
    return nc


def _get_module():
    nc = _STATE.get("nc")
    if nc is None:
        nc = _build_module()
        _STATE["nc"] = nc
    return nc


def _host_prep(bitstrings, theta, idx_pad):
    """Index bookkeeping + input staging. Returns per-core input maps."""
    bitstrings = np.asarray(bitstrings)
    theta = np.asarray(theta, dtype=np.float32)
    idx_pad = np.asarray(idx_pad).astype(np.int64)

    # mask[t] = XOR-fold of one-hot bit positions (pad index >= NUM_BITS -> no bit)
    onehots = np.where(idx_pad >= NUM_BITS, 0, np.int64(1) << np.clip(idx_pad, 0, 62))
    masks = np.bitwise_xor.reduce(onehots, axis=1)
    if masks.size and int(masks.max()) >= KEYS:
        raise NotImplementedError(
            "kernel specialized for masks spanning bits 0..11 "
            f"(max mask {int(masks.max())})"
        )
    theta_spread = np.zeros(KEYS, np.float32)
    np.add.at(theta_spread, masks, theta)

    # Row 32 of bitsT is constant 1; its weight is -j so the key matmuls
    # produce key(b) - j directly.
    wp = np.zeros((NUM_BITS + 1, P_DIM), np.float32)
    for k in range(C_BITS, ORDER):
        wp[k, :] = float(1 << (k - C_BITS))
    wp[NUM_BITS, :] = -np.arange(P_DIM, dtype=np.float32)
    wc = np.zeros((NUM_BITS + 1, C_DIM), np.float32)
    for k in range(C_BITS):
        wc[k, :] = float(1 << k)
    wc[NUM_BITS, :] = -np.arange(C_DIM, dtype=np.float32)

    pk33 = np.zeros((NUM_BITS + 1, PK32_COLS), np.float32)
    pk33[0:C_DIM, 0:128] = theta_spread.reshape(P_DIM, C_DIM).T
    pk33[0:C_DIM, 128:160] = _sylvester(C_DIM)
    pk33[:, 160:288] = wp
    pk33[:, 288:320] = wc
    pk33[0:C_DIM, 321] = 1.0

    base = {"pk33": pk33, "pk128": _sylvester(P_DIM)}

    bits_f = bitstrings.astype(np.float32)
    in_maps = []
    for c in range(N_CORES):
        m = dict(base)
        bt = np.ones((NUM_BITS + 1, B_LOCAL), np.float32)
        bt[:NUM_BITS, :] = bits_f[c * B_LOCAL : (c + 1) * B_LOCAL, :].T
        m["bitst"] = bt
        in_maps.append(m)
    return in_maps


def kernel(bitstrings, theta, idx_pad):
    from concourse.bass_utils import run_bass_kernel_spmd

    in_maps = _host_prep(bitstrings, theta, idx_pad)
    nc = _get_module()
    res = run_bass_kernel_spmd(nc, in_maps, core_ids=list(range(N_CORES)))
    out = np.concatenate([np.asarray(r["out"][0]) for r in res.results])
    return out.astype(np.float32)
